# revision 28
# baseline (speedup 1.0000x reference)
"""Trainium2 Bass kernel for a 7-layer Riptide-style binarized CNN.

Strategy (data-parallel over 8 NeuronCores, 64 images/core):
  - conv1 (full precision) is one K=27 float32r matmul per 450 output
    positions from a host-built im2col matrix, 4 concurrent 32-row PE
    strips via tile_position.  float32r matches the reference conv
    bit-for-bit (threshold margins get as small as 1.6e-8, so any
    reordered/split arithmetic flips signs and corrupts images).
  - Every BN(+relu)(+maxpool)->sign boundary folds into a per-output-
    channel threshold: next layer's +-1 input is Sign(psum + bias) on the
    scalar engine straight out of PSUM.  maxpool commutes with relu and
    monotone BN so pooling runs on raw PSUM counts.
  - conv2..7 operands are +-1 fp8e4m3; PSUM accumulates exact integer
    counts in fp32.
  - Shifted duplicate activation copies (for tap-pair DoubleRow in
    L2/L3) are produced by GpSimd tensor_copy, halving scalar load.
  - L4/L5/L6 (Cin>=256) use fp8 DoubleRow over Cin-halves.  L5/L6 run
    on pad-sharing vertically stacked 4-image grids [37 x 10]: one
    280-col stream covers 4 images' 8x8 outputs (vs 400 full-grid),
    garbage boundary rows discarded by strided PSUM reads.
  - Weights stay resident in SBUF; each weight load is reused across
    2-4 concurrent PSUM accumulators so LDWEIGHTS hides under matmuls.
  - Pad rims are memset to +1 once (sign(0)=+1); interiors rewritten
    per group.  L7 (4x4x512 -> 10) is deferred and batched over all 64
    images, followed by one softmax chain and a single output DMA.
"""

import os
import sys

sys.path.insert(0, "/opt/trn_rl_repo")

import numpy as np
import ml_dtypes
from contextlib import ExitStack

import concourse.bass as bass  # noqa: F401
import concourse.mybir as mybir
import concourse.tile as tile
from concourse import bacc
from concourse.bass_utils import run_bass_kernel_spmd
from concourse.masks import make_identity
from concourse.ap import AP as RawAP

F32 = mybir.dt.float32
BF16 = mybir.dt.bfloat16
FP8 = mybir.dt.float8e4
NP8 = ml_dtypes.float8_e4m3fn
NBF = ml_dtypes.bfloat16
DR = mybir.MatmulPerfMode.DoubleRow

NCORES = 8
B = 512
NB = B // NCORES
EPS = 1e-3
BIG = 1e30

TAPS9 = [(dy, dx) for dy in range(3) for dx in range(3)]

KC = {2: 1, 3: 1, 4: 2, 5: 2, 6: 4}
MC = {2: 1, 3: 2, 4: 2, 5: 4, 6: 4}

_OFF = {}
_o = 0
for _l in (2, 3, 4, 5, 6):
    _OFF[_l] = _o
    _o += 9 * KC[_l] * MC[_l] * 128
_OFF[7] = _o
WTOT = _o + 16 * 4 * 10

CVCOL = {1: 0, 2: 1, 3: 2, 4: 4, 5: 6, 6: 10}

_prog_cache = {}


def build_program(nb=NB, g=32):
    assert nb % g == 0 and g % 4 == 0
    nch = g // 4  # stacked 4-image chunks for L5/L6

    span2 = g * 1156 + 96
    span3 = g * 324 + 48
    span4 = g * 324 + 48

    nc = bacc.Bacc("TRN2", target_bir_lowering=False, debug=False)
    Sign = mybir.ActivationFunctionType.Sign
    Exp = mybir.ActivationFunctionType.Exp
    Identity = mybir.ActivationFunctionType.Identity
    AX = mybir.AxisListType.X

    x1 = nc.declare_dram_parameter("x1", [27, nb * 900 + 10800], F32, isOutput=False)
    w1 = nc.declare_dram_parameter("w1", [128, 128], F32, isOutput=False)
    wall = nc.declare_dram_parameter("wall", [128, WTOT], FP8, isOutput=False)
    cvec = nc.declare_dram_parameter("cvec", [128, 16], F32, isOutput=False)
    wallp = nc.declare_dram_parameter("wallp", [128, 3456], FP8, isOutput=False)
    bn7 = nc.declare_dram_parameter("bn7", [10, 2], F32, isOutput=False)
    y = nc.declare_dram_parameter("y", [nb, 10], F32, isOutput=True)

    with tile.TileContext(nc) as tc, ExitStack() as ctx:
        consts = ctx.enter_context(tc.tile_pool(name="consts", bufs=1))
        sbufs = ctx.enter_context(tc.tile_pool(name="sbufs", bufs=1))
        xpool = ctx.enter_context(tc.tile_pool(name="xpool", bufs=2))
        xbig = ctx.enter_context(tc.tile_pool(name="xbig", bufs=2))
        post = ctx.enter_context(tc.tile_pool(name="post", bufs=4))
        pp = ctx.enter_context(tc.tile_pool(name="pp", bufs=1, space="PSUM"))
        psum7 = ctx.enter_context(tc.tile_pool(name="psum7", bufs=1, space="PSUM"))

        w1sb = consts.tile([128, 128], F32)
        nc.sync.dma_start(out=w1sb, in_=w1[:, :])
        cv = consts.tile([128, 16], F32)
        nc.sync.dma_start(out=cv, in_=cvec[:, :])
        bn7sb = consts.tile([10, 2], F32)
        nc.sync.dma_start(out=bn7sb, in_=bn7[:, :])
        # weight tiles; the DMAs are emitted inside the group-0 L1 loop
        # (below) so the first im2col chunks get full HBM bandwidth.
        wpsb = consts.tile([128, 3456], FP8)
        wsb = consts.tile([128, WTOT], FP8)

        def preload_weights():
            # Activation HWDGE queue; only what the first two phases need
            # (L2/L3 pairs + L4).  wsb[:, 0:_OFF[4]] is never read (L2/L3
            # use wpsb) so it is skipped entirely.
            nc.scalar.dma_start(out=wpsb, in_=wallp[:, :])
            nc.scalar.dma_start(
                out=wsb[:, _OFF[4] : _OFF[5]], in_=wall[:, _OFF[4] : _OFF[5]]
            )

        def preload_weights_late():
            # L5 and L6+L7 blocks (3.6MB) deferred past the startup
            # bandwidth crunch; needed only ~200us into the run.
            nc.scalar.dma_start(
                out=wsb[:, _OFF[5] : _OFF[6]], in_=wall[:, _OFF[5] : _OFF[6]]
            )
            nc.scalar.dma_start(out=wsb[:, _OFF[6] :], in_=wall[:, _OFF[6] :])
        ident = consts.tile([10, 10], F32)
        make_identity(nc, ident)

        # DoubleRow weight views: [128, (tap), (kc), (mc), 128]
        def wview(layer):
            n = 9 * KC[layer] * MC[layer] * 128
            return wsb[:, _OFF[layer] : _OFF[layer] + n].rearrange(
                "p (t k m q) -> p t k m q",
                t=9, k=KC[layer], m=MC[layer], q=128,
            )

        wl4, wl5, wl6 = wview(4), wview(5), wview(6)
        # tap-pair weights: L2 pairs [3,2,128] @0, L2 singles [3,128] @768,
        # L3 pairs [3,2,2,128] @1152, L3 singles [3,2,128] @2688
        w2p = wpsb[:, 0:768].rearrange("p (d j q) -> p d j q", d=3, j=2, q=128)
        w2s = wpsb[:, 768:1152].rearrange("p (d q) -> p d q", d=3, q=128)
        w3p = wpsb[:, 1152:2688].rearrange(
            "p (d j m q) -> p d j m q", d=3, j=2, m=2, q=128
        )
        w3s = wpsb[:, 2688:3456].rearrange(
            "p (d m q) -> p d m q", d=3, m=2, q=128
        )

        # persistent activation buffers (one group's worth, reused)
        # s2f/s3f: single-copy padded sign grids with 16B-aligned row
        # pitch (48 / 32), so a DoubleRow matmul pairs vertically
        # adjacent taps (0,dx)+(1,dx) via a custom AP whose pair-dim
        # stride equals the row pitch (step%16==0 satisfied).
        s2f = sbufs.tile([128, g, 34, 48], FP8)
        s3f = sbufs.tile([128, g, 18, 32], FP8)
        s4f = sbufs.tile([128, 2, span4], FP8)
        s4i = s4f[:, :, : g * 324].rearrange(
            "p k (i y x) -> p k i y x", i=g, y=18, x=18
        )
        # L5/L6 inputs: 4 images stacked vertically with shared pad rows.
        # Rows 0..35 = 4 blocks of 9 (pad row + 8 content rows); row 36 =
        # bottom pad.  Valid conv-window starts for image i are rows
        # 9i..9i+7; rows 8,17,26 are cross-image garbage (discarded).
        s5k = sbufs.tile([128, 2, nch, 37, 10], FP8)
        s6k = sbufs.tile([128, 4, nch, 37, 10], FP8)
        s7a = sbufs.tile([128, 4, nb, 4, 4], FP8)

        # ---- pad rims to +1 (sign(0)=+1); interiors rewritten per group
        _ms_eng = [nc.gpsimd, nc.vector]
        _ms_i = [0]

        def rim(apv):
            _ms_eng[_ms_i[0] % 2].memset(apv, 1.0)
            _ms_i[0] += 1

        rim(s2f[:, :, 0:2, 0:34])
        rim(s2f[:, :, 32:34, 0:34])
        rim(s2f[:, :, 2:32, 0:2])
        rim(s2f[:, :, 2:32, 32:34])
        rim(s3f[:, :, 0:1, 0:18])
        rim(s3f[:, :, 17:18, 0:18])
        rim(s3f[:, :, 1:17, 0:1])
        rim(s3f[:, :, 1:17, 17:18])
        for j in range(2):
            rim(s4i[:, j, :, 0:1, :])
            rim(s4i[:, j, :, 17:18, :])
            rim(s4i[:, j, :, 1:17, 0:1])
            rim(s4i[:, j, :, 1:17, 17:18])
        rim(s4f[:, :, g * 324 :])
        rim(s5k)
        rim(s6k)

        _pq = [0]

        def ptile():
            t = _pq[0] % 6
            _pq[0] += 1
            return pp.tile([128, 512], F32, tag=f"q{t}", name=f"q{t}")

        def mmr(out, lhsT, rhs, start, stop, perf_mode=None):
            # matmul WITHOUT reloading the stationary operand: the PE
            # reuses the weights loaded by the immediately preceding
            # self-loading matmul (j-loops stream several images through
            # identical weights; the redundant ~140ns DR LDWEIGHTS
            # otherwise outpaces the 120ns matmuls and becomes the
            # bottleneck of the 256/280-col layers).
            te = nc.tensor
            keep = {0}
            if perf_mode in (DR, mybir.MatmulPerfMode.DoubleRowSwInterleave):
                keep.add(1)
            ifmap_ap = te.lower_ap(rhs.opt(keep), opt=False)
            weights_ap = te.lower_ap(
                lhsT.opt(keep), opt=False, for_matmul_weights=True
            )
            out_ap = te.lower_ap(out)
            return te.add_instruction(
                mybir.InstMatmult(
                    name=nc.get_next_instruction_name(),
                    replication_resolution=0,
                    replication_shift_amnt=0,
                    replication_num_rows=0,
                    start_tensor_calc=start,
                    stop_tensor_calc=stop,
                    ins=[ifmap_ap, weights_ap],
                    outs=[out_ap],
                    perf_mode=perf_mode,
                    is_transpose=False,
                    ifmap_quant_offset=None,
                    weights_quant_offset=None,
                    bass_skip_group_check=False,
                    tile_position=(0, 0),
                    tile_size=(128, 128),
                    ldweights=False,
                )
            )

        def vpair(v, pitch):
            # [p, rows, cols] -> [p, 2, rows, cols] where the pair dim
            # strides by one grid row (vertical DoubleRow tap pairing)
            a = [list(d) for d in v.ap]
            return RawAP(v.tensor, v.offset, [a[0], [pitch, 2], a[1], a[2]])

        def tbias(layer, mc):
            c = CVCOL[layer] + mc
            return cv[:, c : c + 1]

        p7s = []
        s7v = s7a.rearrange("p k i y x -> p k i (y x)")

        def emit_l7(lgrp):
            # L7 matmuls for group lgrp (scheduled mid-stream so only the
            # last group's L7 + softmax sit in the kernel tail)
            p7g = psum7.tile([10, g], F32, tag=f"p7{lgrp}", name=f"p7{lgrp}")
            p7s.append(p7g)
            k = 0
            for t in range(16):
                for kc in range(4):
                    o = _OFF[7] + (t * 4 + kc) * 10
                    nc.tensor.matmul(
                        p7g, wsb[:, o : o + 10],
                        s7v[:, kc, lgrp * g : lgrp * g + g, t],
                        start=(k == 0), stop=(k == 63),
                    )
                    k += 1

        for grp in range(nb // g):
            i00 = grp * g

            # ---------- L1: conv1 (fp32r, 4x row-tiled strips) + bias/relu/BN1/sign
            # One DMA descriptor per PE strip covers FOUR 4-image chunks
            # (strided src AP) so descriptor issue (~0.7us each) does not
            # pace the L1 phase.  Group 0 starts with 1/1/2-image chunks
            # so the first matmul only waits for one 97KB im2col DMA.

            def l1_img(img, st, half, rhs):
                p = ptile()
                pv = p[:, 0:450].rearrange("p (y x) -> p y x", y=15)
                nc.tensor.matmul(
                    pv, w1sb[32 * st : 32 * st + 27, :], rhs,
                    start=True, stop=True, tile_position=(32 * st, 0),
                )
                r0 = half * 15
                dst = s2f[:, img, 2 + r0 : 17 + r0, 2:32]
                if img % 2 == 0:
                    nc.scalar.activation(
                        dst, pv, Sign, bias=tbias(1, 0), scale=1.0
                    )
                else:
                    # odd images leave conv1 as {0,1} = (z >= -b) on the
                    # vector engine (halves the scalar psum-drain); L2
                    # uses the matching rescaled threshold column.
                    nc.vector.tensor_scalar(
                        dst, pv, cv[:, 14:15], None, mybir.AluOpType.is_ge
                    )

            if grp == 0:
                for c0, csz in [(0, 1), (1, 1), (2, 2)]:
                    xt = xpool.tile([128, 2, 450], F32, tag="xt")
                    base = (i00 + c0) * 900
                    for st in range(csz):
                        nc.sync.dma_start(
                            out=xt[32 * st : 32 * st + 27, :, :].rearrange(
                                "p a b -> p (a b)"
                            ),
                            in_=x1[:, base + st * 900 : base + (st + 1) * 900],
                        )
                    for sc in range(2 * csz):
                        st, half = sc % csz, sc // csz
                        l1_img(c0 + st, st, half,
                               xt[32 * st : 32 * st + 27, half, :])
                    if grp == 0 and c0 == 1:
                        preload_weights()
                blocks = [(4, 3)] + [(16, 4) for _ in range(1)]
            else:
                blocks = [(0, 4), (16, 4)]
            for b0i, nch4 in blocks:
                xb = xbig.tile([128, 4, 2, 450], F32, tag="xb")
                for st in range(4):
                    s0 = (i00 + b0i + st) * 900
                    srcv = x1[:, s0 : s0 + nch4 * 3600].rearrange(
                        "p (c f q) -> p c f q", c=nch4, q=900
                    )[:, :, 0, :]
                    nc.sync.dma_start(
                        out=xb[32 * st : 32 * st + 27, 0:nch4, :, :].rearrange(
                            "p c a b -> p c (a b)"
                        ),
                        in_=srcv,
                    )
                for c in range(nch4):
                    for sc in range(8):
                        st, half = sc % 4, sc // 4
                        img = b0i + 4 * c + st
                        l1_img(img, st, half,
                               xb[32 * st : 32 * st + 27, c, half, :])

            # ---------- L2: binconv 128->128 (vertical tap-pair DR), pool, BN2, sign
            for b0 in range(0, g, 2):
                for rc in range(2):
                    ps = [ptile() for _ in range(2)]
                    for dx in range(3):
                        for j in range(2):
                            rv = s2f[:, b0 + j, rc * 16 : rc * 16 + 16, dx : dx + 32]
                            (nc.tensor.matmul if j == 0 else mmr)(
                                ps[j][:, 0:512], w2p[:, dx, :, :],
                                vpair(rv, 48),
                                start=(dx == 0), stop=False, perf_mode=DR,
                            )
                    for dx in range(3):
                        for j in range(2):
                            (nc.tensor.matmul if j == 0 else mmr)(
                                ps[j][:, 0:512], w2s[:, dx, :],
                                s2f[:, b0 + j, rc * 16 + 2 : rc * 16 + 18, dx : dx + 32],
                                start=False, stop=(dx == 2),
                            )
                    for j in range(2):
                        pv = ps[j][:, 0:512].rearrange("p (y x) -> p y x", y=16)
                        t1 = post.tile([128, 16, 16], F32, tag="t1")
                        nc.vector.reduce_max(
                            t1, pv.rearrange("p y (x two) -> p y x two", two=2),
                            axis=AX,
                        )
                        t2 = post.tile([128, 8, 16], F32, tag="t2")
                        nc.vector.reduce_max(
                            t2, t1.rearrange("p (y two) x -> p y x two", two=2),
                            axis=AX,
                        )
                        b2 = tbias(2, 0) if (b0 + j) % 2 == 0 else cv[:, 15:16]
                        nc.scalar.activation(
                            s3f[:, b0 + j, 1 + rc * 8 : 9 + rc * 8, 1:17], t2,
                            Sign, bias=b2, scale=1.0,
                        )

            if grp == 0:
                preload_weights_late()
            else:
                emit_l7(grp - 1)

            # ---------- L3: binconv 128->256 (vertical tap-pair DR), BN3, pad, sign
            for mc in range(2):
                for b0 in range(0, g, 4):
                    ps = [ptile() for _ in range(4)]
                    for dx in range(3):
                        for j in range(4):
                            rv = s3f[:, b0 + j, 0:16, dx : dx + 16]
                            (nc.tensor.matmul if j == 0 else mmr)(
                                ps[j][:, 0:256], w3p[:, dx, :, mc, :],
                                vpair(rv, 32),
                                start=(dx == 0), stop=False, perf_mode=DR,
                            )
                    for dx in range(3):
                        for j in range(4):
                            (nc.tensor.matmul if j == 0 else mmr)(
                                ps[j][:, 0:256], w3s[:, dx, mc, :],
                                s3f[:, b0 + j, 2:18, dx : dx + 16],
                                start=False, stop=(dx == 2),
                            )
                    for j in range(4):
                        pv = ps[j][:, 0:256].rearrange("p (y x) -> p y x", y=16)
                        nc.scalar.activation(
                            s4i[:, mc, b0 + j, 1:17, 1:17], pv, Sign,
                            bias=tbias(3, mc), scale=1.0,
                        )

            # ---------- L4: binconv 256->256 (DoubleRow), pool, BN4, sign
            for mc in range(2):
                for b0 in range(0, g, 4):
                    ps = [ptile() for _ in range(4)]
                    for t, (dy, dx) in enumerate(TAPS9):
                        for j in range(4):
                            (nc.tensor.matmul if j == 0 else mmr)(
                                ps[j][:, 0:256], wl4[:, t, 0:2, mc, :],
                                s4i[:, :, b0 + j, dy : dy + 16, dx : dx + 16],
                                start=(t == 0), stop=(t == 8), perf_mode=DR,
                            )
                    for j in range(4):
                        pv = ps[j][:, 0:256].rearrange("p (y x) -> p y x", y=16)
                        t1 = post.tile([128, 16, 8], F32, tag="t1")
                        nc.vector.reduce_max(
                            t1, pv.rearrange("p y (x two) -> p y x two", two=2),
                            axis=AX,
                        )
                        t2 = post.tile([128, 8, 8], F32, tag="t2")
                        nc.vector.reduce_max(
                            t2, t1.rearrange("p (y two) x -> p y x two", two=2),
                            axis=AX,
                        )
                        img = b0 + j
                        chk, jj = img // 4, img % 4
                        nc.scalar.activation(
                            s5k[:, mc, chk, 9 * jj + 1 : 9 * jj + 9, 1:9], t2,
                            Sign, bias=tbias(4, mc), scale=1.0,
                        )

            # ---------- L5: binconv 256->512 (DoubleRow, stacked grid), BN5, sign
            for mc in range(4):
                for cb in range(0, nch, 4):
                    ps = [ptile() for _ in range(4)]
                    for t, (dy, dx) in enumerate(TAPS9):
                        for c4 in range(4):
                            (nc.tensor.matmul if c4 == 0 else mmr)(
                                ps[c4][:, 0:280], wl5[:, t, 0:2, mc, :],
                                s5k[:, :, cb + c4, dy : dy + 35, dx : dx + 8],
                                start=(t == 0), stop=(t == 8), perf_mode=DR,
                            )
                    for c4 in range(4):
                        srcv = ps[c4][:, 0:288].rearrange(
                            "p (i r x) -> p i r x", i=4, r=9, x=8
                        )[:, :, 0:8, :]
                        dst = s6k[:, mc, cb + c4, 0:36, :].rearrange(
                            "p (i r) x -> p i r x", i=4, r=9
                        )[:, :, 1:9, 1:9]
                        nc.scalar.activation(dst, srcv, Sign, bias=tbias(5, mc), scale=1.0)

            # ---------- L6: binconv 512->512 (DoubleRow, stacked), pool, BN6, sign
            for mc in range(4):
                for cb in range(0, nch, 4):
                    ps = [ptile() for _ in range(4)]
                    for kp in range(2):
                        for t, (dy, dx) in enumerate(TAPS9):
                            for c4 in range(4):
                                (nc.tensor.matmul if c4 == 0 else mmr)(
                                    ps[c4][:, 0:280], wl6[:, t, 2 * kp : 2 * kp + 2, mc, :],
                                    s6k[:, 2 * kp : 2 * kp + 2, cb + c4, dy : dy + 35, dx : dx + 8],
                                    start=(kp == 0 and t == 0),
                                    stop=(kp == 1 and t == 8), perf_mode=DR,
                                )
                    for c4 in range(4):
                        pv = ps[c4][:, 0:288].rearrange(
                            "p (i r x) -> p i r x", i=4, r=9, x=8
                        )[:, :, 0:8, :]
                        t1 = post.tile([128, 4, 8, 4], F32, tag="t1")
                        nc.vector.reduce_max(
                            t1, pv.rearrange("p i y (x two) -> p i y x two", two=2),
                            axis=AX,
                        )
                        t2 = post.tile([128, 4, 4, 4], F32, tag="t2")
                        nc.vector.reduce_max(
                            t2, t1.rearrange("p i (y two) x -> p i y x two", two=2),
                            axis=AX,
                        )
                        nc.scalar.activation(
                            s7a[:, mc, i00 + 4 * (cb + c4) : i00 + 4 * (cb + c4) + 4, :, :], t2,
                            Sign, bias=tbias(6, mc), scale=1.0,
                        )

        # ---------- BN7 + softmax over all images
        emit_l7(nb // g - 1)
        h7 = post.tile([10, nb], F32, tag="h7")
        for gi, pg in enumerate(p7s):
            nc.vector.tensor_scalar_max(h7[:, gi * g : (gi + 1) * g], pg, 0.0)
        v7 = post.tile([10, nb], F32, tag="v7")
        nc.scalar.activation(
            v7, h7, Identity, bias=bn7sb[:, 1:2], scale=bn7sb[:, 0:1]
        )
        ptt = ptile()
        pt = ptt[0:nb, 0:10]
        nc.tensor.transpose(pt, v7, ident)
        mx = post.tile([nb, 1], F32, tag="mx")
        nc.vector.reduce_max(mx, pt, axis=AX)
        nmx = post.tile([nb, 1], F32, tag="nmx")
        nc.vector.tensor_scalar_mul(nmx, mx, -1.0)
        ex = post.tile([nb, 10], F32, tag="ex")
        nc.scalar.activation(ex, pt, Exp, bias=nmx, scale=1.0)
        sm = post.tile([nb, 1], F32, tag="sm")
        nc.vector.reduce_sum(sm, ex, axis=AX)
        ri = post.tile([nb, 1], F32, tag="ri")
        nc.vector.reciprocal(ri, sm)
        yo = post.tile([nb, 10], F32, tag="yo")
        nc.vector.tensor_scalar_mul(yo, ex, ri)
        nc.sync.dma_start(out=y[:, :], in_=yo)

    nc.compile()
    return nc


# ------------------------------------------------------------------ host prep

def _thresh_bias(gm, be, m, v):
    """bias such that next-layer input = Sign(pre_bn_value + bias)."""
    a = gm.astype(np.float64) / np.sqrt(v.astype(np.float64) + EPS)
    c = be.astype(np.float64) - a * m.astype(np.float64)
    return np.where(c < 0.0, c / a, BIG).astype(np.float32)  # -T = c/a


def _pack_w(wl):
    """sign(w) [3,3,Cin,Cout] -> [128, 9*KC*MC*128] fp8, (tap,kc,mc,q) order."""
    s = np.where(wl >= 0, 1.0, -1.0).astype(np.float32)
    _, _, cin, cout = wl.shape
    kc, mcn = cin // 128, cout // 128
    a = s.reshape(3, 3, kc, 128, mcn, 128)
    a = np.ascontiguousarray(a.transpose(3, 0, 1, 2, 4, 5))
    return a.reshape(128, 9 * kc * mcn * 128).astype(NP8)


def _prep_shared(inputs):
    d = {k: np.asarray(v, np.float32) for k, v in inputs.items()}

    wall = np.empty((128, WTOT), dtype=NP8)
    for layer in (2, 3, 4, 5, 6):
        wl = _pack_w(d[f"w{layer}"])
        wall[:, _OFF[layer] : _OFF[layer] + wl.shape[1]] = wl
    s7w = np.where(d["w7"] >= 0, 1.0, -1.0).astype(np.float32)
    a = s7w.reshape(4, 4, 4, 128, 10).transpose(3, 0, 1, 2, 4)
    wall[:, _OFF[7] :] = np.ascontiguousarray(a).reshape(128, 640).astype(NP8)

    cvec = np.zeros((128, 16), dtype=np.float32)
    tb1 = _thresh_bias(d["g1"], d["be1"], d["m1"], d["v1"])
    cvec[:, 0] = (d["b1"].astype(np.float64) + tb1.astype(np.float64)).astype(
        np.float32
    )
    # col 14: threshold for the vector {0,1} conv1 drain (z >= -b1tot)
    cvec[:, 14] = -cvec[:, 0]
    # col 15: L2 sign threshold in the {0,1} activation domain:
    # count01 = (count_pm + sum(w2)) / 2  =>  b01 = (b_pm - sum(w2)) / 2
    s2w_sum = np.where(d["w2"] >= 0, 1.0, -1.0).sum(axis=(0, 1, 2))  # [128]
    a2 = d["g2"].astype(np.float64) / np.sqrt(d["v2"].astype(np.float64) + EPS)
    c2 = d["be2"].astype(np.float64) - a2 * d["m2"].astype(np.float64)
    tb2_64 = np.where(c2 < 0.0, c2 / a2, BIG)
    cvec[:, 15] = ((tb2_64 - s2w_sum.astype(np.float64)) / 2.0).astype(np.float32)
    for layer in (2, 3, 4, 5, 6):
        tb = _thresh_bias(
            d[f"g{layer}"], d[f"be{layer}"], d[f"m{layer}"], d[f"v{layer}"]
        )
        cvec[:, CVCOL[layer] : CVCOL[layer] + MC[layer]] = tb.reshape(
            MC[layer], 128
        ).T

    a7 = d["g7"].astype(np.float64) / np.sqrt(d["v7"].astype(np.float64) + EPS)
    c7 = d["be7"].astype(np.float64) - a7 * d["m7"].astype(np.float64)
    bn7 = np.stack([a7.astype(np.float32), c7.astype(np.float32)], axis=1)

    wp = np.empty((128, 3456), dtype=NP8)
    s2w = np.where(d["w2"] >= 0, 1.0, -1.0).astype(np.float32)
    s3w = np.where(d["w3"] >= 0, 1.0, -1.0).astype(np.float32)
    for dx in range(3):
        for j in range(2):
            wp[:, (dx * 2 + j) * 128 : (dx * 2 + j + 1) * 128] = s2w[j, dx].astype(NP8)
        wp[:, 768 + dx * 128 : 768 + (dx + 1) * 128] = s2w[2, dx].astype(NP8)
        for j in range(2):
            for m in range(2):
                o = 1152 + ((dx * 2 + j) * 2 + m) * 128
                wp[:, o : o + 128] = s3w[j, dx, :, m * 128 : (m + 1) * 128].astype(NP8)
        for m in range(2):
            o = 2688 + (dx * 2 + m) * 128
            wp[:, o : o + 128] = s3w[2, dx, :, m * 128 : (m + 1) * 128].astype(NP8)

    w1r = np.zeros((128, 128), dtype=np.float32)
    for st in range(4):
        w1r[32 * st : 32 * st + 27, :] = d["w1"].reshape(27, 128)
    return d, wall, wp, cvec, bn7, w1r


def _im2col(x):
    """x [B,32,32,3] -> [27, B, 900] f32, row order (dy,dx,c)."""
    from numpy.lib.stride_tricks import sliding_window_view

    sw = sliding_window_view(x, (3, 3), axis=(1, 2))  # [B,30,30,3,3,3]
    im = sw.transpose(4, 5, 3, 0, 1, 2).reshape(27, x.shape[0], 900)
    return np.ascontiguousarray(im)


LAST_RESULTS = None


def kernel(**inputs):
    global LAST_RESULTS
    nb, g = NB, 32
    key = (nb, g)
    if key not in _prog_cache:
        _prog_cache[key] = build_program(nb, g)
    nc = _prog_cache[key]

    d, wall, wp, cvec, bn7, w1r = _prep_shared(inputs)
    im = _im2col(d["x"])  # [27, B, 900] f32

    in_maps = []
    for c in range(NCORES):
        xi = np.zeros((27, nb * 900 + 10800), dtype=np.float32)
        xi[:, : nb * 900] = im[:, c * nb : (c + 1) * nb, :].reshape(
            27, nb * 900
        )
        in_maps.append(
            {"x1": xi, "w1": w1r, "wall": wall, "wallp": wp, "cvec": cvec,
             "bn7": bn7}
        )

    trace = bool(int(os.environ.get("KERNEL_TRACE", "0")))
    res = run_bass_kernel_spmd(
        nc, in_maps, core_ids=list(range(NCORES)), trace=trace
    )
    LAST_RESULTS = res
    out = np.concatenate([res.results[i]["y"] for i in range(NCORES)], axis=0)
    return out.astype(np.float32)


# revision 29
# speedup vs baseline: 1.0244x; 1.0244x over previous
"""Trainium2 Bass kernel for a 7-layer Riptide-style binarized CNN.

Strategy (data-parallel over 8 NeuronCores, 64 images/core):
  - conv1 (full precision) is one K=27 float32r matmul per 450 output
    positions from a host-built im2col matrix, 4 concurrent 32-row PE
    strips via tile_position.  float32r matches the reference conv
    bit-for-bit (threshold margins get as small as 1.6e-8, so any
    reordered/split arithmetic flips signs and corrupts images).
  - Every BN(+relu)(+maxpool)->sign boundary folds into a per-output-
    channel threshold: next layer's +-1 input is Sign(psum + bias) on the
    scalar engine straight out of PSUM.  maxpool commutes with relu and
    monotone BN so pooling runs on raw PSUM counts.
  - conv2..7 operands are +-1 fp8e4m3; PSUM accumulates exact integer
    counts in fp32.
  - Shifted duplicate activation copies (for tap-pair DoubleRow in
    L2/L3) are produced by GpSimd tensor_copy, halving scalar load.
  - L4/L5/L6 (Cin>=256) use fp8 DoubleRow over Cin-halves.  L5/L6 run
    on pad-sharing vertically stacked 4-image grids [37 x 10]: one
    280-col stream covers 4 images' 8x8 outputs (vs 400 full-grid),
    garbage boundary rows discarded by strided PSUM reads.
  - Weights stay resident in SBUF; each weight load is reused across
    2-4 concurrent PSUM accumulators so LDWEIGHTS hides under matmuls.
  - Pad rims are memset to +1 once (sign(0)=+1); interiors rewritten
    per group.  L7 (4x4x512 -> 10) is deferred and batched over all 64
    images, followed by one softmax chain and a single output DMA.
"""

import os
import sys

sys.path.insert(0, "/opt/trn_rl_repo")

import numpy as np
import ml_dtypes
from contextlib import ExitStack

import concourse.bass as bass  # noqa: F401
import concourse.mybir as mybir
import concourse.tile as tile
from concourse import bacc
from concourse.bass_utils import run_bass_kernel_spmd
from concourse.masks import make_identity
from concourse.ap import AP as RawAP

F32 = mybir.dt.float32
BF16 = mybir.dt.bfloat16
FP8 = mybir.dt.float8e4
NP8 = ml_dtypes.float8_e4m3fn
NBF = ml_dtypes.bfloat16
DR = mybir.MatmulPerfMode.DoubleRow

NCORES = 8
B = 512
NB = B // NCORES
EPS = 1e-3
BIG = 1e30

TAPS9 = [(dy, dx) for dy in range(3) for dx in range(3)]

KC = {2: 1, 3: 1, 4: 2, 5: 2, 6: 4}
MC = {2: 1, 3: 2, 4: 2, 5: 4, 6: 4}

_OFF = {}
_o = 0
for _l in (2, 3, 4, 5, 6):
    _OFF[_l] = _o
    _o += 9 * KC[_l] * MC[_l] * 128
_OFF[7] = _o
WTOT = _o + 16 * 4 * 10

CVCOL = {1: 0, 2: 1, 3: 2, 4: 4, 5: 6, 6: 10}

_prog_cache = {}


def build_program(nb=NB, g=32):
    assert nb % g == 0 and g % 4 == 0
    nch = g // 4  # stacked 4-image chunks for L5/L6

    span2 = g * 1156 + 96
    span3 = g * 324 + 48
    span4 = g * 324 + 48

    nc = bacc.Bacc("TRN2", target_bir_lowering=False, debug=False)
    Sign = mybir.ActivationFunctionType.Sign
    Exp = mybir.ActivationFunctionType.Exp
    Identity = mybir.ActivationFunctionType.Identity
    AX = mybir.AxisListType.X

    x1 = nc.declare_dram_parameter("x1", [27, nb * 900 + 10800], F32, isOutput=False)
    w1 = nc.declare_dram_parameter("w1", [128, 128], F32, isOutput=False)
    wall = nc.declare_dram_parameter("wall", [128, WTOT], FP8, isOutput=False)
    cvec = nc.declare_dram_parameter("cvec", [128, 16], F32, isOutput=False)
    wallp = nc.declare_dram_parameter("wallp", [128, 3456], FP8, isOutput=False)
    bn7 = nc.declare_dram_parameter("bn7", [10, 2], F32, isOutput=False)
    y = nc.declare_dram_parameter("y", [nb, 10], F32, isOutput=True)

    with tile.TileContext(nc) as tc, ExitStack() as ctx:
        consts = ctx.enter_context(tc.tile_pool(name="consts", bufs=1))
        sbufs = ctx.enter_context(tc.tile_pool(name="sbufs", bufs=1))
        xpool = ctx.enter_context(tc.tile_pool(name="xpool", bufs=6))
        post = ctx.enter_context(tc.tile_pool(name="post", bufs=4))
        pp = ctx.enter_context(tc.tile_pool(name="pp", bufs=1, space="PSUM"))
        psum7 = ctx.enter_context(tc.tile_pool(name="psum7", bufs=1, space="PSUM"))

        w1sb = consts.tile([128, 128], F32)
        nc.sync.dma_start(out=w1sb, in_=w1[:, :])
        cv = consts.tile([128, 16], F32)
        nc.sync.dma_start(out=cv, in_=cvec[:, :])
        bn7sb = consts.tile([10, 2], F32)
        nc.sync.dma_start(out=bn7sb, in_=bn7[:, :])
        # weight tiles; the DMAs are emitted inside the group-0 L1 loop
        # (below) so the first im2col chunks get full HBM bandwidth.
        wpsb = consts.tile([128, 3456], FP8)
        wsb = consts.tile([128, WTOT], FP8)

        def preload_weights():
            # Activation HWDGE queue; only what the first two phases need
            # (L2/L3 pairs + L4).  wsb[:, 0:_OFF[4]] is never read (L2/L3
            # use wpsb) so it is skipped entirely.
            nc.scalar.dma_start(out=wpsb, in_=wallp[:, :])
            nc.scalar.dma_start(
                out=wsb[:, _OFF[4] : _OFF[5]], in_=wall[:, _OFF[4] : _OFF[5]]
            )

        def preload_weights_late():
            # L5 and L6+L7 blocks (3.6MB) deferred past the startup
            # bandwidth crunch; needed only ~200us into the run.
            nc.scalar.dma_start(
                out=wsb[:, _OFF[5] : _OFF[6]], in_=wall[:, _OFF[5] : _OFF[6]]
            )
            nc.scalar.dma_start(out=wsb[:, _OFF[6] :], in_=wall[:, _OFF[6] :])
        ident = consts.tile([10, 10], F32)
        make_identity(nc, ident)

        # DoubleRow weight views: [128, (tap), (kc), (mc), 128]
        def wview(layer):
            n = 9 * KC[layer] * MC[layer] * 128
            return wsb[:, _OFF[layer] : _OFF[layer] + n].rearrange(
                "p (t k m q) -> p t k m q",
                t=9, k=KC[layer], m=MC[layer], q=128,
            )

        wl4, wl5, wl6 = wview(4), wview(5), wview(6)
        # tap-pair weights: L2 pairs [3,2,128] @0, L2 singles [3,128] @768,
        # L3 pairs [3,2,2,128] @1152, L3 singles [3,2,128] @2688
        w2p = wpsb[:, 0:768].rearrange("p (d j q) -> p d j q", d=3, j=2, q=128)
        w2s = wpsb[:, 768:1152].rearrange("p (d q) -> p d q", d=3, q=128)
        w3p = wpsb[:, 1152:2688].rearrange(
            "p (d j m q) -> p d j m q", d=3, j=2, m=2, q=128
        )
        w3s = wpsb[:, 2688:3456].rearrange(
            "p (d m q) -> p d m q", d=3, m=2, q=128
        )

        # persistent activation buffers (one group's worth, reused)
        # s2f/s3f: single-copy padded sign grids with 16B-aligned row
        # pitch (48 / 32), so a DoubleRow matmul pairs vertically
        # adjacent taps (0,dx)+(1,dx) via a custom AP whose pair-dim
        # stride equals the row pitch (step%16==0 satisfied).
        s2f = sbufs.tile([128, g, 34, 48], FP8)
        s3f = sbufs.tile([128, g, 18, 32], FP8)
        s4f = sbufs.tile([128, 2, span4], FP8)
        s4i = s4f[:, :, : g * 324].rearrange(
            "p k (i y x) -> p k i y x", i=g, y=18, x=18
        )
        # L5/L6 inputs: 4 images stacked vertically with shared pad rows.
        # Rows 0..35 = 4 blocks of 9 (pad row + 8 content rows); row 36 =
        # bottom pad.  Valid conv-window starts for image i are rows
        # 9i..9i+7; rows 8,17,26 are cross-image garbage (discarded).
        s5k = sbufs.tile([128, 2, nch, 37, 10], FP8)
        s6k = sbufs.tile([128, 4, nch, 37, 10], FP8)
        s7a = sbufs.tile([128, 4, nb, 4, 4], FP8)

        # ---- pad rims to +1 (sign(0)=+1); interiors rewritten per group
        _ms_eng = [nc.gpsimd, nc.vector]
        _ms_i = [0]

        def rim(apv):
            _ms_eng[_ms_i[0] % 2].memset(apv, 1.0)
            _ms_i[0] += 1

        rim(s2f[:, :, 0:2, 0:34])
        rim(s2f[:, :, 32:34, 0:34])
        rim(s2f[:, :, 2:32, 0:2])
        rim(s2f[:, :, 2:32, 32:34])
        rim(s3f[:, :, 0:1, 0:18])
        rim(s3f[:, :, 17:18, 0:18])
        rim(s3f[:, :, 1:17, 0:1])
        rim(s3f[:, :, 1:17, 17:18])
        for j in range(2):
            rim(s4i[:, j, :, 0:1, :])
            rim(s4i[:, j, :, 17:18, :])
            rim(s4i[:, j, :, 1:17, 0:1])
            rim(s4i[:, j, :, 1:17, 17:18])
        rim(s4f[:, :, g * 324 :])
        rim(s5k)
        rim(s6k)

        _pq = [0]

        def ptile():
            t = _pq[0] % 6
            _pq[0] += 1
            return pp.tile([128, 512], F32, tag=f"q{t}", name=f"q{t}")

        def mmr(out, lhsT, rhs, start, stop, perf_mode=None):
            # matmul WITHOUT reloading the stationary operand: the PE
            # reuses the weights loaded by the immediately preceding
            # self-loading matmul (j-loops stream several images through
            # identical weights; the redundant ~140ns DR LDWEIGHTS
            # otherwise outpaces the 120ns matmuls and becomes the
            # bottleneck of the 256/280-col layers).
            te = nc.tensor
            keep = {0}
            if perf_mode in (DR, mybir.MatmulPerfMode.DoubleRowSwInterleave):
                keep.add(1)
            ifmap_ap = te.lower_ap(rhs.opt(keep), opt=False)
            weights_ap = te.lower_ap(
                lhsT.opt(keep), opt=False, for_matmul_weights=True
            )
            out_ap = te.lower_ap(out)
            return te.add_instruction(
                mybir.InstMatmult(
                    name=nc.get_next_instruction_name(),
                    replication_resolution=0,
                    replication_shift_amnt=0,
                    replication_num_rows=0,
                    start_tensor_calc=start,
                    stop_tensor_calc=stop,
                    ins=[ifmap_ap, weights_ap],
                    outs=[out_ap],
                    perf_mode=perf_mode,
                    is_transpose=False,
                    ifmap_quant_offset=None,
                    weights_quant_offset=None,
                    bass_skip_group_check=False,
                    tile_position=(0, 0),
                    tile_size=(128, 128),
                    ldweights=False,
                )
            )

        def vpair(v, pitch):
            # [p, rows, cols] -> [p, 2, rows, cols] where the pair dim
            # strides by one grid row (vertical DoubleRow tap pairing)
            a = [list(d) for d in v.ap]
            return RawAP(v.tensor, v.offset, [a[0], [pitch, 2], a[1], a[2]])

        def tbias(layer, mc):
            c = CVCOL[layer] + mc
            return cv[:, c : c + 1]

        p7s = []
        s7v = s7a.rearrange("p k i y x -> p k i (y x)")

        def emit_l7(lgrp):
            # L7 matmuls for group lgrp (scheduled mid-stream so only the
            # last group's L7 + softmax sit in the kernel tail)
            p7g = psum7.tile([10, g], F32, tag=f"p7{lgrp}", name=f"p7{lgrp}")
            p7s.append(p7g)
            k = 0
            for t in range(16):
                for kc in range(4):
                    o = _OFF[7] + (t * 4 + kc) * 10
                    nc.tensor.matmul(
                        p7g, wsb[:, o : o + 10],
                        s7v[:, kc, lgrp * g : lgrp * g + g, t],
                        start=(k == 0), stop=(k == 63),
                    )
                    k += 1

        for grp in range(nb // g):
            i00 = grp * g

            # ---------- L1: conv1 (fp32r, 4x row-tiled strips) + bias/relu/BN1/sign
            # One DMA descriptor per PE strip covers FOUR 4-image chunks
            # (strided src AP) so descriptor issue (~0.7us each) does not
            # pace the L1 phase.  Group 0 starts with 1/1/2-image chunks
            # so the first matmul only waits for one 97KB im2col DMA.

            def l1_img(img, st, half, rhs):
                p = ptile()
                pv = p[:, 0:450].rearrange("p (y x) -> p y x", y=15)
                nc.tensor.matmul(
                    pv, w1sb[32 * st : 32 * st + 27, :], rhs,
                    start=True, stop=True, tile_position=(32 * st, 0),
                )
                r0 = half * 15
                dst = s2f[:, img, 2 + r0 : 17 + r0, 2:32]
                if img % 2 == 0:
                    nc.scalar.activation(
                        dst, pv, Sign, bias=tbias(1, 0), scale=1.0
                    )
                else:
                    # odd images leave conv1 as {0,1} = (z >= -b) on the
                    # vector engine (halves the scalar psum-drain); L2
                    # uses the matching rescaled threshold column.
                    nc.vector.tensor_scalar(
                        dst, pv, cv[:, 14:15], None, mybir.AluOpType.is_ge
                    )

            if grp == 0:
                for c0, csz in [(0, 1), (1, 1), (2, 2)]:
                    xt = xpool.tile([128, 2, 450], F32, tag="xt")
                    base = (i00 + c0) * 900
                    for st in range(csz):
                        nc.sync.dma_start(
                            out=xt[32 * st : 32 * st + 27, :, :].rearrange(
                                "p a b -> p (a b)"
                            ),
                            in_=x1[:, base + st * 900 : base + (st + 1) * 900],
                        )
                    for sc in range(2 * csz):
                        st, half = sc % csz, sc // csz
                        l1_img(c0 + st, st, half,
                               xt[32 * st : 32 * st + 27, half, :])
                    if grp == 0 and c0 == 1:
                        preload_weights()
                rest = [(c, 4) for c in range(4, g, 4)]
            else:
                rest = [(c, 4) for c in range(0, g, 4)]
            for c0, csz in rest:
                xt = xpool.tile([128, 2, 450], F32, tag="xt")
                base = (i00 + c0) * 900
                for st in range(csz):
                    nc.sync.dma_start(
                        out=xt[32 * st : 32 * st + 27, :, :].rearrange(
                            "p a b -> p (a b)"
                        ),
                        in_=x1[:, base + st * 900 : base + (st + 1) * 900],
                    )
                for sc in range(2 * csz):
                    st, half = sc % csz, sc // csz
                    l1_img(c0 + st, st, half,
                           xt[32 * st : 32 * st + 27, half, :])

            # ---------- L2: binconv 128->128 (vertical tap-pair DR), pool, BN2, sign
            for b0 in range(0, g, 2):
                for rc in range(2):
                    ps = [ptile() for _ in range(2)]
                    for dx in range(3):
                        for j in range(2):
                            rv = s2f[:, b0 + j, rc * 16 : rc * 16 + 16, dx : dx + 32]
                            (nc.tensor.matmul if j == 0 else mmr)(
                                ps[j][:, 0:512], w2p[:, dx, :, :],
                                vpair(rv, 48),
                                start=(dx == 0), stop=False, perf_mode=DR,
                            )
                    for dx in range(3):
                        for j in range(2):
                            (nc.tensor.matmul if j == 0 else mmr)(
                                ps[j][:, 0:512], w2s[:, dx, :],
                                s2f[:, b0 + j, rc * 16 + 2 : rc * 16 + 18, dx : dx + 32],
                                start=False, stop=(dx == 2),
                            )
                    for j in range(2):
                        pv = ps[j][:, 0:512].rearrange("p (y x) -> p y x", y=16)
                        t1 = post.tile([128, 16, 16], F32, tag="t1")
                        nc.vector.reduce_max(
                            t1, pv.rearrange("p y (x two) -> p y x two", two=2),
                            axis=AX,
                        )
                        t2 = post.tile([128, 8, 16], F32, tag="t2")
                        nc.vector.reduce_max(
                            t2, t1.rearrange("p (y two) x -> p y x two", two=2),
                            axis=AX,
                        )
                        b2 = tbias(2, 0) if (b0 + j) % 2 == 0 else cv[:, 15:16]
                        nc.scalar.activation(
                            s3f[:, b0 + j, 1 + rc * 8 : 9 + rc * 8, 1:17], t2,
                            Sign, bias=b2, scale=1.0,
                        )

            if grp == 0:
                preload_weights_late()
            else:
                emit_l7(grp - 1)

            # ---------- L3: binconv 128->256 (vertical tap-pair DR), BN3, pad, sign
            for mc in range(2):
                for b0 in range(0, g, 4):
                    ps = [ptile() for _ in range(4)]
                    for dx in range(3):
                        for j in range(4):
                            rv = s3f[:, b0 + j, 0:16, dx : dx + 16]
                            (nc.tensor.matmul if j == 0 else mmr)(
                                ps[j][:, 0:256], w3p[:, dx, :, mc, :],
                                vpair(rv, 32),
                                start=(dx == 0), stop=False, perf_mode=DR,
                            )
                    for dx in range(3):
                        for j in range(4):
                            (nc.tensor.matmul if j == 0 else mmr)(
                                ps[j][:, 0:256], w3s[:, dx, mc, :],
                                s3f[:, b0 + j, 2:18, dx : dx + 16],
                                start=False, stop=(dx == 2),
                            )
                    for j in range(4):
                        pv = ps[j][:, 0:256].rearrange("p (y x) -> p y x", y=16)
                        nc.scalar.activation(
                            s4i[:, mc, b0 + j, 1:17, 1:17], pv, Sign,
                            bias=tbias(3, mc), scale=1.0,
                        )

            # ---------- L4: binconv 256->256 (DoubleRow), pool, BN4, sign
            for mc in range(2):
                for b0 in range(0, g, 4):
                    ps = [ptile() for _ in range(4)]
                    for t, (dy, dx) in enumerate(TAPS9):
                        for j in range(4):
                            (nc.tensor.matmul if j == 0 else mmr)(
                                ps[j][:, 0:256], wl4[:, t, 0:2, mc, :],
                                s4i[:, :, b0 + j, dy : dy + 16, dx : dx + 16],
                                start=(t == 0), stop=(t == 8), perf_mode=DR,
                            )
                    for j in range(4):
                        pv = ps[j][:, 0:256].rearrange("p (y x) -> p y x", y=16)
                        t1 = post.tile([128, 16, 8], F32, tag="t1")
                        nc.vector.reduce_max(
                            t1, pv.rearrange("p y (x two) -> p y x two", two=2),
                            axis=AX,
                        )
                        t2 = post.tile([128, 8, 8], F32, tag="t2")
                        nc.vector.reduce_max(
                            t2, t1.rearrange("p (y two) x -> p y x two", two=2),
                            axis=AX,
                        )
                        img = b0 + j
                        chk, jj = img // 4, img % 4
                        nc.scalar.activation(
                            s5k[:, mc, chk, 9 * jj + 1 : 9 * jj + 9, 1:9], t2,
                            Sign, bias=tbias(4, mc), scale=1.0,
                        )

            # ---------- L5: binconv 256->512 (DoubleRow, stacked grid), BN5, sign
            for mc in range(4):
                for cb in range(0, nch, 4):
                    ps = [ptile() for _ in range(4)]
                    for t, (dy, dx) in enumerate(TAPS9):
                        for c4 in range(4):
                            (nc.tensor.matmul if c4 == 0 else mmr)(
                                ps[c4][:, 0:280], wl5[:, t, 0:2, mc, :],
                                s5k[:, :, cb + c4, dy : dy + 35, dx : dx + 8],
                                start=(t == 0), stop=(t == 8), perf_mode=DR,
                            )
                    for c4 in range(4):
                        srcv = ps[c4][:, 0:288].rearrange(
                            "p (i r x) -> p i r x", i=4, r=9, x=8
                        )[:, :, 0:8, :]
                        dst = s6k[:, mc, cb + c4, 0:36, :].rearrange(
                            "p (i r) x -> p i r x", i=4, r=9
                        )[:, :, 1:9, 1:9]
                        nc.scalar.activation(dst, srcv, Sign, bias=tbias(5, mc), scale=1.0)

            # ---------- L6: binconv 512->512 (DoubleRow, stacked), pool, BN6, sign
            for mc in range(4):
                for cb in range(0, nch, 4):
                    ps = [ptile() for _ in range(4)]
                    for kp in range(2):
                        for t, (dy, dx) in enumerate(TAPS9):
                            for c4 in range(4):
                                (nc.tensor.matmul if c4 == 0 else mmr)(
                                    ps[c4][:, 0:280], wl6[:, t, 2 * kp : 2 * kp + 2, mc, :],
                                    s6k[:, 2 * kp : 2 * kp + 2, cb + c4, dy : dy + 35, dx : dx + 8],
                                    start=(kp == 0 and t == 0),
                                    stop=(kp == 1 and t == 8), perf_mode=DR,
                                )
                    for c4 in range(4):
                        pv = ps[c4][:, 0:288].rearrange(
                            "p (i r x) -> p i r x", i=4, r=9, x=8
                        )[:, :, 0:8, :]
                        t1 = post.tile([128, 4, 8, 4], F32, tag="t1")
                        nc.vector.reduce_max(
                            t1, pv.rearrange("p i y (x two) -> p i y x two", two=2),
                            axis=AX,
                        )
                        t2 = post.tile([128, 4, 4, 4], F32, tag="t2")
                        nc.vector.reduce_max(
                            t2, t1.rearrange("p i (y two) x -> p i y x two", two=2),
                            axis=AX,
                        )
                        nc.scalar.activation(
                            s7a[:, mc, i00 + 4 * (cb + c4) : i00 + 4 * (cb + c4) + 4, :, :], t2,
                            Sign, bias=tbias(6, mc), scale=1.0,
                        )

        # ---------- BN7 + softmax over all images
        emit_l7(nb // g - 1)
        h7 = post.tile([10, nb], F32, tag="h7")
        for gi, pg in enumerate(p7s):
            nc.vector.tensor_scalar_max(h7[:, gi * g : (gi + 1) * g], pg, 0.0)
        v7 = post.tile([10, nb], F32, tag="v7")
        nc.scalar.activation(
            v7, h7, Identity, bias=bn7sb[:, 1:2], scale=bn7sb[:, 0:1]
        )
        ptt = ptile()
        pt = ptt[0:nb, 0:10]
        nc.tensor.transpose(pt, v7, ident)
        mx = post.tile([nb, 1], F32, tag="mx")
        nc.vector.reduce_max(mx, pt, axis=AX)
        nmx = post.tile([nb, 1], F32, tag="nmx")
        nc.vector.tensor_scalar_mul(nmx, mx, -1.0)
        ex = post.tile([nb, 10], F32, tag="ex")
        nc.scalar.activation(ex, pt, Exp, bias=nmx, scale=1.0)
        sm = post.tile([nb, 1], F32, tag="sm")
        nc.vector.reduce_sum(sm, ex, axis=AX)
        ri = post.tile([nb, 1], F32, tag="ri")
        nc.vector.reciprocal(ri, sm)
        yo = post.tile([nb, 10], F32, tag="yo")
        nc.vector.tensor_scalar_mul(yo, ex, ri)
        nc.sync.dma_start(out=y[:, :], in_=yo)

    nc.compile()
    return nc


# ------------------------------------------------------------------ host prep

def _thresh_bias(gm, be, m, v):
    """bias such that next-layer input = Sign(pre_bn_value + bias)."""
    a = gm.astype(np.float64) / np.sqrt(v.astype(np.float64) + EPS)
    c = be.astype(np.float64) - a * m.astype(np.float64)
    return np.where(c < 0.0, c / a, BIG).astype(np.float32)  # -T = c/a


def _pack_w(wl):
    """sign(w) [3,3,Cin,Cout] -> [128, 9*KC*MC*128] fp8, (tap,kc,mc,q) order."""
    s = np.where(wl >= 0, 1.0, -1.0).astype(np.float32)
    _, _, cin, cout = wl.shape
    kc, mcn = cin // 128, cout // 128
    a = s.reshape(3, 3, kc, 128, mcn, 128)
    a = np.ascontiguousarray(a.transpose(3, 0, 1, 2, 4, 5))
    return a.reshape(128, 9 * kc * mcn * 128).astype(NP8)


def _prep_shared(inputs):
    d = {k: np.asarray(v, np.float32) for k, v in inputs.items()}

    wall = np.empty((128, WTOT), dtype=NP8)
    for layer in (2, 3, 4, 5, 6):
        wl = _pack_w(d[f"w{layer}"])
        wall[:, _OFF[layer] : _OFF[layer] + wl.shape[1]] = wl
    s7w = np.where(d["w7"] >= 0, 1.0, -1.0).astype(np.float32)
    a = s7w.reshape(4, 4, 4, 128, 10).transpose(3, 0, 1, 2, 4)
    wall[:, _OFF[7] :] = np.ascontiguousarray(a).reshape(128, 640).astype(NP8)

    cvec = np.zeros((128, 16), dtype=np.float32)
    tb1 = _thresh_bias(d["g1"], d["be1"], d["m1"], d["v1"])
    cvec[:, 0] = (d["b1"].astype(np.float64) + tb1.astype(np.float64)).astype(
        np.float32
    )
    # col 14: threshold for the vector {0,1} conv1 drain (z >= -b1tot)
    cvec[:, 14] = -cvec[:, 0]
    # col 15: L2 sign threshold in the {0,1} activation domain:
    # count01 = (count_pm + sum(w2)) / 2  =>  b01 = (b_pm - sum(w2)) / 2
    s2w_sum = np.where(d["w2"] >= 0, 1.0, -1.0).sum(axis=(0, 1, 2))  # [128]
    a2 = d["g2"].astype(np.float64) / np.sqrt(d["v2"].astype(np.float64) + EPS)
    c2 = d["be2"].astype(np.float64) - a2 * d["m2"].astype(np.float64)
    tb2_64 = np.where(c2 < 0.0, c2 / a2, BIG)
    cvec[:, 15] = ((tb2_64 - s2w_sum.astype(np.float64)) / 2.0).astype(np.float32)
    for layer in (2, 3, 4, 5, 6):
        tb = _thresh_bias(
            d[f"g{layer}"], d[f"be{layer}"], d[f"m{layer}"], d[f"v{layer}"]
        )
        cvec[:, CVCOL[layer] : CVCOL[layer] + MC[layer]] = tb.reshape(
            MC[layer], 128
        ).T

    a7 = d["g7"].astype(np.float64) / np.sqrt(d["v7"].astype(np.float64) + EPS)
    c7 = d["be7"].astype(np.float64) - a7 * d["m7"].astype(np.float64)
    bn7 = np.stack([a7.astype(np.float32), c7.astype(np.float32)], axis=1)

    wp = np.empty((128, 3456), dtype=NP8)
    s2w = np.where(d["w2"] >= 0, 1.0, -1.0).astype(np.float32)
    s3w = np.where(d["w3"] >= 0, 1.0, -1.0).astype(np.float32)
    for dx in range(3):
        for j in range(2):
            wp[:, (dx * 2 + j) * 128 : (dx * 2 + j + 1) * 128] = s2w[j, dx].astype(NP8)
        wp[:, 768 + dx * 128 : 768 + (dx + 1) * 128] = s2w[2, dx].astype(NP8)
        for j in range(2):
            for m in range(2):
                o = 1152 + ((dx * 2 + j) * 2 + m) * 128
                wp[:, o : o + 128] = s3w[j, dx, :, m * 128 : (m + 1) * 128].astype(NP8)
        for m in range(2):
            o = 2688 + (dx * 2 + m) * 128
            wp[:, o : o + 128] = s3w[2, dx, :, m * 128 : (m + 1) * 128].astype(NP8)

    w1r = np.zeros((128, 128), dtype=np.float32)
    for st in range(4):
        w1r[32 * st : 32 * st + 27, :] = d["w1"].reshape(27, 128)
    return d, wall, wp, cvec, bn7, w1r


def _im2col(x):
    """x [B,32,32,3] -> [27, B, 900] f32, row order (dy,dx,c)."""
    from numpy.lib.stride_tricks import sliding_window_view

    sw = sliding_window_view(x, (3, 3), axis=(1, 2))  # [B,30,30,3,3,3]
    im = sw.transpose(4, 5, 3, 0, 1, 2).reshape(27, x.shape[0], 900)
    return np.ascontiguousarray(im)


LAST_RESULTS = None


def kernel(**inputs):
    global LAST_RESULTS
    nb, g = NB, 32
    key = (nb, g)
    if key not in _prog_cache:
        _prog_cache[key] = build_program(nb, g)
    nc = _prog_cache[key]

    d, wall, wp, cvec, bn7, w1r = _prep_shared(inputs)
    im = _im2col(d["x"])  # [27, B, 900] f32

    in_maps = []
    for c in range(NCORES):
        xi = np.zeros((27, nb * 900 + 10800), dtype=np.float32)
        xi[:, : nb * 900] = im[:, c * nb : (c + 1) * nb, :].reshape(
            27, nb * 900
        )
        in_maps.append(
            {"x1": xi, "w1": w1r, "wall": wall, "wallp": wp, "cvec": cvec,
             "bn7": bn7}
        )

    trace = bool(int(os.environ.get("KERNEL_TRACE", "0")))
    res = run_bass_kernel_spmd(
        nc, in_maps, core_ids=list(range(NCORES)), trace=trace
    )
    LAST_RESULTS = res
    out = np.concatenate([res.results[i]["y"] for i in range(NCORES)], axis=0)
    return out.astype(np.float32)


# revision 30
# speedup vs baseline: 1.0300x; 1.0055x over previous
"""Trainium2 Bass kernel for a 7-layer Riptide-style binarized CNN.

Strategy (data-parallel over 8 NeuronCores, 64 images/core):
  - conv1 (full precision) is one K=27 float32r matmul per 450 output
    positions from a host-built im2col matrix, 4 concurrent 32-row PE
    strips via tile_position.  float32r matches the reference conv
    bit-for-bit (threshold margins get as small as 1.6e-8, so any
    reordered/split arithmetic flips signs and corrupts images).
  - Every BN(+relu)(+maxpool)->sign boundary folds into a per-output-
    channel threshold: next layer's +-1 input is Sign(psum + bias) on the
    scalar engine straight out of PSUM.  maxpool commutes with relu and
    monotone BN so pooling runs on raw PSUM counts.
  - conv2..7 operands are +-1 fp8e4m3; PSUM accumulates exact integer
    counts in fp32.
  - L2/L3 pair vertically adjacent taps (0,dx)+(1,dx) in one DoubleRow
    stream via custom APs whose pair-dim stride equals the (16B-aligned)
    row pitch -- no duplicated shifted activation copies needed.
  - conv1's PSUM drain alternates between the scalar engine (Sign, +-1)
    and the vector engine (is_ge, {0,1} domain with rescaled L2
    thresholds for odd images), so neither engine paces the L1 phase.
  - L4/L5/L6 (Cin>=256) use fp8 DoubleRow over Cin-halves.  L5/L6 run
    on pad-sharing vertically stacked 4-image grids [37 x 10]: one
    280-col stream covers 4 images' 8x8 outputs (vs 400 full-grid),
    garbage boundary rows discarded by strided PSUM reads.
  - Weights stay resident in SBUF; each weight load is reused across
    2-4 concurrent PSUM accumulators so LDWEIGHTS hides under matmuls.
  - Pad rims are memset to +1 once (sign(0)=+1); interiors rewritten
    per group.  L7 (4x4x512 -> 10) is deferred and batched over all 64
    images, followed by one softmax chain and a single output DMA.
"""

import os
import sys

sys.path.insert(0, "/opt/trn_rl_repo")

import numpy as np
import ml_dtypes
from contextlib import ExitStack

import concourse.bass as bass  # noqa: F401
import concourse.mybir as mybir
import concourse.tile as tile
from concourse import bacc
from concourse.bass_utils import run_bass_kernel_spmd
from concourse.masks import make_identity
from concourse.ap import AP as RawAP

F32 = mybir.dt.float32
BF16 = mybir.dt.bfloat16
FP8 = mybir.dt.float8e4
NP8 = ml_dtypes.float8_e4m3fn
NBF = ml_dtypes.bfloat16
DR = mybir.MatmulPerfMode.DoubleRow

NCORES = 8
B = 512
NB = B // NCORES
EPS = 1e-3
BIG = 1e30

TAPS9 = [(dy, dx) for dy in range(3) for dx in range(3)]

KC = {2: 1, 3: 1, 4: 2, 5: 2, 6: 4}
MC = {2: 1, 3: 2, 4: 2, 5: 4, 6: 4}

_OFF = {}
_o = 0
for _l in (2, 3, 4, 5, 6):
    _OFF[_l] = _o
    _o += 9 * KC[_l] * MC[_l] * 128
_OFF[7] = _o
WTOT = _o + 16 * 4 * 10

CVCOL = {1: 0, 2: 1, 3: 2, 4: 4, 5: 6, 6: 10}

_prog_cache = {}


def build_program(nb=NB, g=32):
    assert nb % g == 0 and g % 4 == 0
    nch = g // 4  # stacked 4-image chunks for L5/L6

    span2 = g * 1156 + 96
    span3 = g * 324 + 48
    span4 = g * 324 + 48

    nc = bacc.Bacc("TRN2", target_bir_lowering=False, debug=False)
    Sign = mybir.ActivationFunctionType.Sign
    Exp = mybir.ActivationFunctionType.Exp
    Identity = mybir.ActivationFunctionType.Identity
    AX = mybir.AxisListType.X

    x1 = nc.declare_dram_parameter("x1", [27, nb * 900 + 10800], F32, isOutput=False)
    w1 = nc.declare_dram_parameter("w1", [128, 128], F32, isOutput=False)
    wall = nc.declare_dram_parameter("wall", [128, WTOT], FP8, isOutput=False)
    cvec = nc.declare_dram_parameter("cvec", [128, 16], F32, isOutput=False)
    wallp = nc.declare_dram_parameter("wallp", [128, 3456], FP8, isOutput=False)
    bn7 = nc.declare_dram_parameter("bn7", [10, 2], F32, isOutput=False)
    y = nc.declare_dram_parameter("y", [nb, 10], F32, isOutput=True)

    with tile.TileContext(nc) as tc, ExitStack() as ctx:
        consts = ctx.enter_context(tc.tile_pool(name="consts", bufs=1))
        sbufs = ctx.enter_context(tc.tile_pool(name="sbufs", bufs=1))
        xpool = ctx.enter_context(tc.tile_pool(name="xpool", bufs=6))
        post = ctx.enter_context(tc.tile_pool(name="post", bufs=4))
        pp = ctx.enter_context(tc.tile_pool(name="pp", bufs=1, space="PSUM"))
        psum7 = ctx.enter_context(tc.tile_pool(name="psum7", bufs=1, space="PSUM"))

        w1sb = consts.tile([128, 128], F32)
        nc.sync.dma_start(out=w1sb, in_=w1[:, :])
        cv = consts.tile([128, 16], F32)
        nc.sync.dma_start(out=cv, in_=cvec[:, :])
        bn7sb = consts.tile([10, 2], F32)
        nc.sync.dma_start(out=bn7sb, in_=bn7[:, :])
        # weight tiles; the DMAs are emitted inside the group-0 L1 loop
        # (below) so the first im2col chunks get full HBM bandwidth.
        wpsb = consts.tile([128, 3456], FP8)
        wsb = consts.tile([128, WTOT], FP8)

        def preload_weights():
            # Activation HWDGE queue; only what the first two phases need
            # (L2/L3 pairs + L4).  wsb[:, 0:_OFF[4]] is never read (L2/L3
            # use wpsb) so it is skipped entirely.
            nc.scalar.dma_start(out=wpsb, in_=wallp[:, :])
            nc.scalar.dma_start(
                out=wsb[:, _OFF[4] : _OFF[5]], in_=wall[:, _OFF[4] : _OFF[5]]
            )

        def preload_weights_late():
            # L5 and L6+L7 blocks (3.6MB) deferred past the startup
            # bandwidth crunch; needed only ~200us into the run.
            nc.scalar.dma_start(
                out=wsb[:, _OFF[5] : _OFF[6]], in_=wall[:, _OFF[5] : _OFF[6]]
            )
            nc.scalar.dma_start(out=wsb[:, _OFF[6] :], in_=wall[:, _OFF[6] :])
        ident = consts.tile([10, 10], F32)
        make_identity(nc, ident)

        # DoubleRow weight views: [128, (tap), (kc), (mc), 128]
        def wview(layer):
            n = 9 * KC[layer] * MC[layer] * 128
            return wsb[:, _OFF[layer] : _OFF[layer] + n].rearrange(
                "p (t k m q) -> p t k m q",
                t=9, k=KC[layer], m=MC[layer], q=128,
            )

        wl4, wl5, wl6 = wview(4), wview(5), wview(6)
        # tap-pair weights: L2 pairs [3,2,128] @0, L2 singles [3,128] @768,
        # L3 pairs [3,2,2,128] @1152, L3 singles [3,2,128] @2688
        w2p = wpsb[:, 0:768].rearrange("p (d j q) -> p d j q", d=3, j=2, q=128)
        w2s = wpsb[:, 768:1152].rearrange("p (d q) -> p d q", d=3, q=128)
        w3p = wpsb[:, 1152:2688].rearrange(
            "p (d j m q) -> p d j m q", d=3, j=2, m=2, q=128
        )
        w3s = wpsb[:, 2688:3456].rearrange(
            "p (d m q) -> p d m q", d=3, m=2, q=128
        )

        # persistent activation buffers (one group's worth, reused)
        # s2f/s3f: single-copy padded sign grids with 16B-aligned row
        # pitch (48 / 32), so a DoubleRow matmul pairs vertically
        # adjacent taps (0,dx)+(1,dx) via a custom AP whose pair-dim
        # stride equals the row pitch (step%16==0 satisfied).
        s2f = sbufs.tile([128, g, 34, 48], FP8)
        s3f = sbufs.tile([128, g, 18, 32], FP8)
        s4f = sbufs.tile([128, 2, span4], FP8)
        s4i = s4f[:, :, : g * 324].rearrange(
            "p k (i y x) -> p k i y x", i=g, y=18, x=18
        )
        # L5/L6 inputs: 4 images stacked vertically with shared pad rows.
        # Rows 0..35 = 4 blocks of 9 (pad row + 8 content rows); row 36 =
        # bottom pad.  Valid conv-window starts for image i are rows
        # 9i..9i+7; rows 8,17,26 are cross-image garbage (discarded).
        s5k = sbufs.tile([128, 2, nch, 37, 10], FP8)
        s6k = sbufs.tile([128, 4, nch, 37, 10], FP8)
        s7a = sbufs.tile([128, 4, nb, 4, 4], FP8)

        # ---- pad rims to +1 (sign(0)=+1); interiors rewritten per group
        _ms_eng = [nc.gpsimd, nc.vector]
        _ms_i = [0]

        def rim(apv):
            _ms_eng[_ms_i[0] % 2].memset(apv, 1.0)
            _ms_i[0] += 1

        rim(s2f[:, :, 0:2, 0:34])
        rim(s2f[:, :, 32:34, 0:34])
        rim(s2f[:, :, 2:32, 0:2])
        rim(s2f[:, :, 2:32, 32:34])
        rim(s3f[:, :, 0:1, 0:18])
        rim(s3f[:, :, 17:18, 0:18])
        rim(s3f[:, :, 1:17, 0:1])
        rim(s3f[:, :, 1:17, 17:18])
        for j in range(2):
            rim(s4i[:, j, :, 0:1, :])
            rim(s4i[:, j, :, 17:18, :])
            rim(s4i[:, j, :, 1:17, 0:1])
            rim(s4i[:, j, :, 1:17, 17:18])
        rim(s4f[:, :, g * 324 :])
        rim(s5k)
        rim(s6k)

        _pq = [0]

        def ptile():
            t = _pq[0] % 6
            _pq[0] += 1
            return pp.tile([128, 512], F32, tag=f"q{t}", name=f"q{t}")

        def mmr(out, lhsT, rhs, start, stop, perf_mode=None):
            # matmul WITHOUT reloading the stationary operand: the PE
            # reuses the weights loaded by the immediately preceding
            # self-loading matmul (j-loops stream several images through
            # identical weights; the redundant ~140ns DR LDWEIGHTS
            # otherwise outpaces the 120ns matmuls and becomes the
            # bottleneck of the 256/280-col layers).
            te = nc.tensor
            keep = {0}
            if perf_mode in (DR, mybir.MatmulPerfMode.DoubleRowSwInterleave):
                keep.add(1)
            ifmap_ap = te.lower_ap(rhs.opt(keep), opt=False)
            weights_ap = te.lower_ap(
                lhsT.opt(keep), opt=False, for_matmul_weights=True
            )
            out_ap = te.lower_ap(out)
            return te.add_instruction(
                mybir.InstMatmult(
                    name=nc.get_next_instruction_name(),
                    replication_resolution=0,
                    replication_shift_amnt=0,
                    replication_num_rows=0,
                    start_tensor_calc=start,
                    stop_tensor_calc=stop,
                    ins=[ifmap_ap, weights_ap],
                    outs=[out_ap],
                    perf_mode=perf_mode,
                    is_transpose=False,
                    ifmap_quant_offset=None,
                    weights_quant_offset=None,
                    bass_skip_group_check=False,
                    tile_position=(0, 0),
                    tile_size=(128, 128),
                    ldweights=False,
                )
            )

        def vpair(v, pitch):
            # [p, rows, cols] -> [p, 2, rows, cols] where the pair dim
            # strides by one grid row (vertical DoubleRow tap pairing)
            a = [list(d) for d in v.ap]
            return RawAP(v.tensor, v.offset, [a[0], [pitch, 2], a[1], a[2]])

        def tbias(layer, mc):
            c = CVCOL[layer] + mc
            return cv[:, c : c + 1]

        p7s = []
        s7v = s7a.rearrange("p k i y x -> p k i (y x)")

        def emit_l7(lgrp):
            # L7 matmuls for group lgrp (scheduled mid-stream so only the
            # last group's L7 + softmax sit in the kernel tail)
            p7g = psum7.tile([10, g], F32, tag=f"p7{lgrp}", name=f"p7{lgrp}")
            p7s.append(p7g)
            k = 0
            for t in range(16):
                for kc in range(4):
                    o = _OFF[7] + (t * 4 + kc) * 10
                    nc.tensor.matmul(
                        p7g, wsb[:, o : o + 10],
                        s7v[:, kc, lgrp * g : lgrp * g + g, t],
                        start=(k == 0), stop=(k == 63),
                    )
                    k += 1

        for grp in range(nb // g):
            i00 = grp * g

            # ---------- L1: conv1 (fp32r, 4x row-tiled strips) + bias/relu/BN1/sign
            # One DMA descriptor per PE strip covers FOUR 4-image chunks
            # (strided src AP) so descriptor issue (~0.7us each) does not
            # pace the L1 phase.  Group 0 starts with 1/1/2-image chunks
            # so the first matmul only waits for one 97KB im2col DMA.

            def l1_img(img, st, half, rhs):
                p = ptile()
                pv = p[:, 0:450].rearrange("p (y x) -> p y x", y=15)
                nc.tensor.matmul(
                    pv, w1sb[32 * st : 32 * st + 27, :], rhs,
                    start=True, stop=True, tile_position=(32 * st, 0),
                )
                r0 = half * 15
                dst = s2f[:, img, 2 + r0 : 17 + r0, 2:32]
                if img % 2 == 0:
                    nc.scalar.activation(
                        dst, pv, Sign, bias=tbias(1, 0), scale=1.0
                    )
                else:
                    # odd images leave conv1 as {0,1} = (z >= -b) on the
                    # vector engine (halves the scalar psum-drain); L2
                    # uses the matching rescaled threshold column.
                    nc.vector.tensor_scalar(
                        dst, pv, cv[:, 14:15], None, mybir.AluOpType.is_ge
                    )

            if grp == 0:
                for c0, csz in [(0, 1), (1, 1), (2, 2)]:
                    xt = xpool.tile([128, 2, 450], F32, tag="xt")
                    base = (i00 + c0) * 900
                    for st in range(csz):
                        nc.sync.dma_start(
                            out=xt[32 * st : 32 * st + 27, :, :].rearrange(
                                "p a b -> p (a b)"
                            ),
                            in_=x1[:, base + st * 900 : base + (st + 1) * 900],
                        )
                    for sc in range(2 * csz):
                        st, half = sc % csz, sc // csz
                        l1_img(c0 + st, st, half,
                               xt[32 * st : 32 * st + 27, half, :])
                    if grp == 0 and c0 == 1:
                        preload_weights()
                rest = [(c, 4) for c in range(4, g, 4)]
            else:
                rest = [(c, 4) for c in range(0, g, 4)]
            for c0, csz in rest:
                xt = xpool.tile([128, 2, 450], F32, tag="xt")
                base = (i00 + c0) * 900
                for st in range(csz):
                    nc.sync.dma_start(
                        out=xt[32 * st : 32 * st + 27, :, :].rearrange(
                            "p a b -> p (a b)"
                        ),
                        in_=x1[:, base + st * 900 : base + (st + 1) * 900],
                    )
                for sc in range(2 * csz):
                    st, half = sc % csz, sc // csz
                    l1_img(c0 + st, st, half,
                           xt[32 * st : 32 * st + 27, half, :])

            # ---------- L2: binconv 128->128 (vertical tap-pair DR), pool, BN2, sign
            for b0 in range(0, g, 2):
                for rc in range(2):
                    ps = [ptile() for _ in range(2)]
                    for dx in range(3):
                        for j in range(2):
                            rv = s2f[:, b0 + j, rc * 16 : rc * 16 + 16, dx : dx + 32]
                            (nc.tensor.matmul if j == 0 else mmr)(
                                ps[j][:, 0:512], w2p[:, dx, :, :],
                                vpair(rv, 48),
                                start=(dx == 0), stop=False, perf_mode=DR,
                            )
                    for dx in range(3):
                        for j in range(2):
                            (nc.tensor.matmul if j == 0 else mmr)(
                                ps[j][:, 0:512], w2s[:, dx, :],
                                s2f[:, b0 + j, rc * 16 + 2 : rc * 16 + 18, dx : dx + 32],
                                start=False, stop=(dx == 2),
                            )
                    for j in range(2):
                        pv = ps[j][:, 0:512].rearrange("p (y x) -> p y x", y=16)
                        t1 = post.tile([128, 16, 16], F32, tag="t1")
                        nc.vector.reduce_max(
                            t1, pv.rearrange("p y (x two) -> p y x two", two=2),
                            axis=AX,
                        )
                        t2 = post.tile([128, 8, 16], F32, tag="t2")
                        nc.vector.reduce_max(
                            t2, t1.rearrange("p (y two) x -> p y x two", two=2),
                            axis=AX,
                        )
                        b2 = tbias(2, 0) if (b0 + j) % 2 == 0 else cv[:, 15:16]
                        nc.scalar.activation(
                            s3f[:, b0 + j, 1 + rc * 8 : 9 + rc * 8, 1:17], t2,
                            Sign, bias=b2, scale=1.0,
                        )

            if grp == 0:
                preload_weights_late()
            else:
                emit_l7(grp - 1)

            # ---------- L3: binconv 128->256 (vertical tap-pair DR), BN3, pad, sign
            for mc in range(2):
                for b0 in range(0, g, 4):
                    ps = [ptile() for _ in range(4)]
                    for dx in range(3):
                        for j in range(4):
                            rv = s3f[:, b0 + j, 0:16, dx : dx + 16]
                            (nc.tensor.matmul if j == 0 else mmr)(
                                ps[j][:, 0:256], w3p[:, dx, :, mc, :],
                                vpair(rv, 32),
                                start=(dx == 0), stop=False, perf_mode=DR,
                            )
                    for dx in range(3):
                        for j in range(4):
                            (nc.tensor.matmul if j == 0 else mmr)(
                                ps[j][:, 0:256], w3s[:, dx, mc, :],
                                s3f[:, b0 + j, 2:18, dx : dx + 16],
                                start=False, stop=(dx == 2),
                            )
                    for j in range(4):
                        pv = ps[j][:, 0:256].rearrange("p (y x) -> p y x", y=16)
                        nc.scalar.activation(
                            s4i[:, mc, b0 + j, 1:17, 1:17], pv, Sign,
                            bias=tbias(3, mc), scale=1.0,
                        )

            # ---------- L4: binconv 256->256 (DoubleRow), pool, BN4, sign
            for mc in range(2):
                for b0 in range(0, g, 4):
                    ps = [ptile() for _ in range(4)]
                    for t, (dy, dx) in enumerate(TAPS9):
                        for j in range(4):
                            (nc.tensor.matmul if j == 0 else mmr)(
                                ps[j][:, 0:256], wl4[:, t, 0:2, mc, :],
                                s4i[:, :, b0 + j, dy : dy + 16, dx : dx + 16],
                                start=(t == 0), stop=(t == 8), perf_mode=DR,
                            )
                    for j in range(4):
                        pv = ps[j][:, 0:256].rearrange("p (y x) -> p y x", y=16)
                        t1 = post.tile([128, 16, 8], F32, tag="t1")
                        nc.vector.reduce_max(
                            t1, pv.rearrange("p y (x two) -> p y x two", two=2),
                            axis=AX,
                        )
                        t2 = post.tile([128, 8, 8], F32, tag="t2")
                        nc.vector.reduce_max(
                            t2, t1.rearrange("p (y two) x -> p y x two", two=2),
                            axis=AX,
                        )
                        img = b0 + j
                        chk, jj = img // 4, img % 4
                        nc.scalar.activation(
                            s5k[:, mc, chk, 9 * jj + 1 : 9 * jj + 9, 1:9], t2,
                            Sign, bias=tbias(4, mc), scale=1.0,
                        )

            # ---------- L5: binconv 256->512 (DoubleRow, stacked grid), BN5, sign
            for mc in range(4):
                for cb in range(0, nch, 4):
                    ps = [ptile() for _ in range(4)]
                    for t, (dy, dx) in enumerate(TAPS9):
                        for c4 in range(4):
                            (nc.tensor.matmul if c4 == 0 else mmr)(
                                ps[c4][:, 0:280], wl5[:, t, 0:2, mc, :],
                                s5k[:, :, cb + c4, dy : dy + 35, dx : dx + 8],
                                start=(t == 0), stop=(t == 8), perf_mode=DR,
                            )
                    for c4 in range(4):
                        srcv = ps[c4][:, 0:288].rearrange(
                            "p (i r x) -> p i r x", i=4, r=9, x=8
                        )[:, :, 0:8, :]
                        dst = s6k[:, mc, cb + c4, 0:36, :].rearrange(
                            "p (i r) x -> p i r x", i=4, r=9
                        )[:, :, 1:9, 1:9]
                        nc.scalar.activation(dst, srcv, Sign, bias=tbias(5, mc), scale=1.0)

            # ---------- L6: binconv 512->512 (DoubleRow, stacked), pool, BN6, sign
            for mc in range(4):
                for cb in range(0, nch, 4):
                    ps = [ptile() for _ in range(4)]
                    for kp in range(2):
                        for t, (dy, dx) in enumerate(TAPS9):
                            for c4 in range(4):
                                (nc.tensor.matmul if c4 == 0 else mmr)(
                                    ps[c4][:, 0:280], wl6[:, t, 2 * kp : 2 * kp + 2, mc, :],
                                    s6k[:, 2 * kp : 2 * kp + 2, cb + c4, dy : dy + 35, dx : dx + 8],
                                    start=(kp == 0 and t == 0),
                                    stop=(kp == 1 and t == 8), perf_mode=DR,
                                )
                    for c4 in range(4):
                        pv = ps[c4][:, 0:288].rearrange(
                            "p (i r x) -> p i r x", i=4, r=9, x=8
                        )[:, :, 0:8, :]
                        t1 = post.tile([128, 4, 8, 4], F32, tag="t1")
                        nc.vector.reduce_max(
                            t1, pv.rearrange("p i y (x two) -> p i y x two", two=2),
                            axis=AX,
                        )
                        t2 = post.tile([128, 4, 4, 4], F32, tag="t2")
                        nc.vector.reduce_max(
                            t2, t1.rearrange("p i (y two) x -> p i y x two", two=2),
                            axis=AX,
                        )
                        nc.scalar.activation(
                            s7a[:, mc, i00 + 4 * (cb + c4) : i00 + 4 * (cb + c4) + 4, :, :], t2,
                            Sign, bias=tbias(6, mc), scale=1.0,
                        )

        # ---------- BN7 + softmax over all images
        emit_l7(nb // g - 1)
        h7 = post.tile([10, nb], F32, tag="h7")
        for gi, pg in enumerate(p7s):
            nc.vector.tensor_scalar_max(h7[:, gi * g : (gi + 1) * g], pg, 0.0)
        v7 = post.tile([10, nb], F32, tag="v7")
        nc.scalar.activation(
            v7, h7, Identity, bias=bn7sb[:, 1:2], scale=bn7sb[:, 0:1]
        )
        ptt = ptile()
        pt = ptt[0:nb, 0:10]
        nc.tensor.transpose(pt, v7, ident)
        mx = post.tile([nb, 1], F32, tag="mx")
        nc.vector.reduce_max(mx, pt, axis=AX)
        nmx = post.tile([nb, 1], F32, tag="nmx")
        nc.vector.tensor_scalar_mul(nmx, mx, -1.0)
        ex = post.tile([nb, 10], F32, tag="ex")
        nc.scalar.activation(ex, pt, Exp, bias=nmx, scale=1.0)
        sm = post.tile([nb, 1], F32, tag="sm")
        nc.vector.reduce_sum(sm, ex, axis=AX)
        ri = post.tile([nb, 1], F32, tag="ri")
        nc.vector.reciprocal(ri, sm)
        yo = post.tile([nb, 10], F32, tag="yo")
        nc.vector.tensor_scalar_mul(yo, ex, ri)
        nc.sync.dma_start(out=y[:, :], in_=yo)

    nc.compile()
    return nc


# ------------------------------------------------------------------ host prep

def _thresh_bias(gm, be, m, v):
    """bias such that next-layer input = Sign(pre_bn_value + bias)."""
    a = gm.astype(np.float64) / np.sqrt(v.astype(np.float64) + EPS)
    c = be.astype(np.float64) - a * m.astype(np.float64)
    return np.where(c < 0.0, c / a, BIG).astype(np.float32)  # -T = c/a


def _pack_w(wl):
    """sign(w) [3,3,Cin,Cout] -> [128, 9*KC*MC*128] fp8, (tap,kc,mc,q) order."""
    s = np.where(wl >= 0, 1.0, -1.0).astype(np.float32)
    _, _, cin, cout = wl.shape
    kc, mcn = cin // 128, cout // 128
    a = s.reshape(3, 3, kc, 128, mcn, 128)
    a = np.ascontiguousarray(a.transpose(3, 0, 1, 2, 4, 5))
    return a.reshape(128, 9 * kc * mcn * 128).astype(NP8)


def _prep_shared(inputs):
    d = {k: np.asarray(v, np.float32) for k, v in inputs.items()}

    wall = np.empty((128, WTOT), dtype=NP8)
    for layer in (2, 3, 4, 5, 6):
        wl = _pack_w(d[f"w{layer}"])
        wall[:, _OFF[layer] : _OFF[layer] + wl.shape[1]] = wl
    s7w = np.where(d["w7"] >= 0, 1.0, -1.0).astype(np.float32)
    a = s7w.reshape(4, 4, 4, 128, 10).transpose(3, 0, 1, 2, 4)
    wall[:, _OFF[7] :] = np.ascontiguousarray(a).reshape(128, 640).astype(NP8)

    cvec = np.zeros((128, 16), dtype=np.float32)
    tb1 = _thresh_bias(d["g1"], d["be1"], d["m1"], d["v1"])
    cvec[:, 0] = (d["b1"].astype(np.float64) + tb1.astype(np.float64)).astype(
        np.float32
    )
    # col 14: threshold for the vector {0,1} conv1 drain (z >= -b1tot)
    cvec[:, 14] = -cvec[:, 0]
    # col 15: L2 sign threshold in the {0,1} activation domain:
    # count01 = (count_pm + sum(w2)) / 2  =>  b01 = (b_pm - sum(w2)) / 2
    s2w_sum = np.where(d["w2"] >= 0, 1.0, -1.0).sum(axis=(0, 1, 2))  # [128]
    a2 = d["g2"].astype(np.float64) / np.sqrt(d["v2"].astype(np.float64) + EPS)
    c2 = d["be2"].astype(np.float64) - a2 * d["m2"].astype(np.float64)
    tb2_64 = np.where(c2 < 0.0, c2 / a2, BIG)
    cvec[:, 15] = ((tb2_64 - s2w_sum.astype(np.float64)) / 2.0).astype(np.float32)
    for layer in (2, 3, 4, 5, 6):
        tb = _thresh_bias(
            d[f"g{layer}"], d[f"be{layer}"], d[f"m{layer}"], d[f"v{layer}"]
        )
        cvec[:, CVCOL[layer] : CVCOL[layer] + MC[layer]] = tb.reshape(
            MC[layer], 128
        ).T

    a7 = d["g7"].astype(np.float64) / np.sqrt(d["v7"].astype(np.float64) + EPS)
    c7 = d["be7"].astype(np.float64) - a7 * d["m7"].astype(np.float64)
    bn7 = np.stack([a7.astype(np.float32), c7.astype(np.float32)], axis=1)

    wp = np.empty((128, 3456), dtype=NP8)
    s2w = np.where(d["w2"] >= 0, 1.0, -1.0).astype(np.float32)
    s3w = np.where(d["w3"] >= 0, 1.0, -1.0).astype(np.float32)
    for dx in range(3):
        for j in range(2):
            wp[:, (dx * 2 + j) * 128 : (dx * 2 + j + 1) * 128] = s2w[j, dx].astype(NP8)
        wp[:, 768 + dx * 128 : 768 + (dx + 1) * 128] = s2w[2, dx].astype(NP8)
        for j in range(2):
            for m in range(2):
                o = 1152 + ((dx * 2 + j) * 2 + m) * 128
                wp[:, o : o + 128] = s3w[j, dx, :, m * 128 : (m + 1) * 128].astype(NP8)
        for m in range(2):
            o = 2688 + (dx * 2 + m) * 128
            wp[:, o : o + 128] = s3w[2, dx, :, m * 128 : (m + 1) * 128].astype(NP8)

    w1r = np.zeros((128, 128), dtype=np.float32)
    for st in range(4):
        w1r[32 * st : 32 * st + 27, :] = d["w1"].reshape(27, 128)
    return d, wall, wp, cvec, bn7, w1r


def _im2col(x):
    """x [B,32,32,3] -> [27, B, 900] f32, row order (dy,dx,c)."""
    from numpy.lib.stride_tricks import sliding_window_view

    sw = sliding_window_view(x, (3, 3), axis=(1, 2))  # [B,30,30,3,3,3]
    im = sw.transpose(4, 5, 3, 0, 1, 2).reshape(27, x.shape[0], 900)
    return np.ascontiguousarray(im)


LAST_RESULTS = None


def kernel(**inputs):
    global LAST_RESULTS
    nb, g = NB, 32
    key = (nb, g)
    if key not in _prog_cache:
        _prog_cache[key] = build_program(nb, g)
    nc = _prog_cache[key]

    d, wall, wp, cvec, bn7, w1r = _prep_shared(inputs)
    im = _im2col(d["x"])  # [27, B, 900] f32

    in_maps = []
    for c in range(NCORES):
        xi = np.zeros((27, nb * 900 + 10800), dtype=np.float32)
        xi[:, : nb * 900] = im[:, c * nb : (c + 1) * nb, :].reshape(
            27, nb * 900
        )
        in_maps.append(
            {"x1": xi, "w1": w1r, "wall": wall, "wallp": wp, "cvec": cvec,
             "bn7": bn7}
        )

    trace = bool(int(os.environ.get("KERNEL_TRACE", "0")))
    res = run_bass_kernel_spmd(
        nc, in_maps, core_ids=list(range(NCORES)), trace=trace
    )
    LAST_RESULTS = res
    out = np.concatenate([res.results[i]["y"] for i in range(NCORES)], axis=0)
    return out.astype(np.float32)
